# revision 52
# baseline (speedup 1.0000x reference)
"""Multi-head attention (B=16, N=512, H=8, D=128) on 8 trn2 NeuronCores.

Data-parallel over batch: each core handles 2 batches. Per core:
  qT/kT projections in [d, token] layout (fp32r matmuls, N=512 -> 1 cyc/row),
  scores computed transposed sT[m, n] so the attention*V matmul needs no
  transposes and softmax denominators come from PE ones-matmuls.
  exp(s + dist + colmask) is factored as exp(s) * E with E = exp(distT + cm)
  computed once per batch (shared across all 8 heads) -> per-(b,h) elementwise
  work is one ACT exp pass + one DVE bf16 2x multiply pass.
  The v-bias is folded into the output bias on the host (softmax rows sum to
  1 exactly): bo' = bo + Wo^T bv.  Softmax normalization and the final row
  mask fold into the output projection: out = sum_h Wo_h^T (yraw_h * rinvm_h)
  + bo' (x) mask_row, with rinvm = mask / rowsum.
"""

import sys

sys.path.insert(0, "/opt/trn_rl_repo")

import numpy as np
from contextlib import ExitStack

import ml_dtypes
import concourse.bass as bass
import concourse.bacc as bacc
import concourse.tile as tile
from concourse import mybir
from concourse.masks import make_identity

B, N, H, D = 16, 512, 8, 128
NCORES = 8
BPC = B // NCORES  # batches per core
NT = N // 128  # 128-token tiles per batch
F32 = mybir.dt.float32
F32R = mybir.dt.float32r
BF16 = mybir.dt.bfloat16


def r(ap):
    """reinterpret an fp32 AP as float32r for full-rate PE matmuls"""
    return ap.bitcast(F32R)


def bcastP(ap_1d, p):
    """broadcast a 1-d DRAM AP across p partitions"""
    return bass.AP(tensor=ap_1d.tensor, offset=ap_1d.offset, ap=[[0, p]] + ap_1d.ap)


def build_kernel():
    nc = bacc.Bacc("TRN2", target_bir_lowering=False, debug=False)

    # packed inputs (minimize DMA count: each DMA issue serializes ~0.65us on
    # the shared HWDGE generator)
    #   xm_in  [BPC, 128, 516] f32: cols 0-511 x as [p, nt, d]; 512-515 maskT
    #   wb_in  [128, 4096] bf16: wq' | wk | wv | wo(k-major, head, d_out)
    #   wf_in  [128, 144] f32: cols 0-7 bq', 8-15 bk, 16-143 row0 = bo_eff
    xm_d = nc.declare_dram_parameter("xm_in", [BPC, 128, 516], F32, isOutput=False).ap()
    dc_d = nc.declare_dram_parameter("dc_in", [BPC, 128, NT * N], BF16, isOutput=False).ap()
    mask_d = nc.declare_dram_parameter("mask_in", [BPC, N], F32R, isOutput=False).ap()
    wb_d = nc.declare_dram_parameter("wb_in", [D, 4 * H * D], BF16, isOutput=False).ap()
    wf_d = nc.declare_dram_parameter("wf_in", [D, 144], F32R, isOutput=False).ap()
    y_d = nc.declare_dram_parameter("y_out", [BPC, N, D], F32, isOutput=True).ap()

    rinv_scratch = nc.dram_tensor("rinv_scratch", [BPC, H, N], BF16).ap()

    HH = 4  # heads per pipeline half

    with tile.TileContext(nc) as tc, ExitStack() as ctx:
        # ---------------- pools ----------------
        consts = ctx.enter_context(tc.tile_pool(name="consts", bufs=1))
        stage = ctx.enter_context(tc.tile_pool(name="stage", bufs=2))
        dnat = ctx.enter_context(tc.tile_pool(name="dnat", bufs=2))
        qkp = ctx.enter_context(tc.tile_pool(name="qkp", bufs=8))
        vpool = ctx.enter_context(tc.tile_pool(name="vpool", bufs=8))
        xpool = ctx.enter_context(tc.tile_pool(name="xpool", bufs=3))
        ppool = ctx.enter_context(tc.tile_pool(name="ppool", bufs=16))
        ypool = ctx.enter_context(tc.tile_pool(name="ypool", bufs=6))
        rpool = ctx.enter_context(tc.tile_pool(name="rpool", bufs=4))

        # PSUM budget (8 banks): shared main pool 4 + pst 2 + pso 1 + rs 1
        ps_a = ctx.enter_context(tc.tile_pool(name="ps_main", bufs=4, space="PSUM"))
        ps_y = ps_a
        ps_t = ctx.enter_context(tc.tile_pool(name="ps_t", bufs=2, space="PSUM"))
        ps_rs = ctx.enter_context(tc.tile_pool(name="ps_rs", bufs=1, space="PSUM"))

        # ---------------- prefetch in priority order: x0, weights, dist, x1 ----------------
        xm0 = stage.tile([128, 516], F32, tag="xm", name="xm0")
        nc.sync.dma_start(out=xm0, in_=xm_d[0])
        wb = consts.tile([128, 4 * H * D], BF16, tag="wb")
        nc.sync.dma_start(out=wb, in_=wb_d)
        dc0 = dnat.tile([128, NT * N], BF16, tag="dnat", name="dc0")
        nc.sync.dma_start(out=dc0, in_=dc_d[0])
        xm1 = stage.tile([128, 516], F32, tag="xm", name="xm1")
        nc.sync.dma_start(out=xm1, in_=xm_d[1])
        wf = consts.tile([128, 144], F32R, tag="wf")
        nc.sync.dma_start(out=wf, in_=wf_d)
        dc1 = dnat.tile([128, NT * N], BF16, tag="dnat", name="dc1")
        nc.sync.dma_start(out=dc1, in_=dc_d[1])
        xms = [xm0, xm1]
        dcs = [dc0, dc1]

        wq_sb = wb[:, 0:1024]
        wk_sb = wb[:, 1024:2048]
        wv_sb = wb[:, 2048:3072]
        wo_sb = wb[:, 3072:4096].rearrange("k (h d) -> k h d", h=H)
        bq_sb = wf[:, 0:8].bitcast(F32)
        bo_sb = wf[0:1, 16:144]
        ident = consts.tile([128, 128], F32, tag="ident")
        make_identity(nc, ident)
        ones_bf = consts.tile([128, 1], BF16, tag="ones")
        nc.vector.memset(ones_bf, 1.0)
        warm = consts.tile([128, 512], BF16, tag="warm")
        nc.vector.memset(warm, 0.0)

        # HAM warmup: keep the PE clock gate open while DMAs land
        for w in range(10):
            psw = ps_a.tile([128, N], F32, tag="ps_a", name=f"warm{w}")
            nc.tensor.matmul(psw[0:1, :], ones_bf, warm)

        fronts = []
        for b in range(BPC):
            # ---------------- x + maskT (prefetched), mask4 (1 DMA) ----------------
            xm = xms[b]
            x_nat = xm[:, 0:512].rearrange("p (t d) -> p t d", t=NT)
            mask4 = stage.tile([HH, N], F32R, tag="mask4")
            nc.sync.dma_start(out=mask4, in_=bcastP(mask_d[b], HH))

            # ---------------- x transpose: xT [d, n] (bf16) ----------------
            xT = xpool.tile([128, N], BF16, tag="xT")
            pstx = ps_t.tile([128, 512], F32, tag="pst", name=f"pstx{b}")
            for nt in range(NT):
                nc.tensor.transpose(pstx[:, nt * 128:(nt + 1) * 128], x_nat[:, nt, :], ident)
            nc.vector.tensor_copy(out=xT, in_=pstx)

            # ---------------- v projection -> v[mt] [m, d_all] (bias folded out) ----------------
            vv = []
            for mt in range(NT):
                vmt = vpool.tile([128, H * D], BF16, tag="vv", name=f"v{b}_{mt}")
                for dh in range(2):
                    psv = ps_a.tile([128, N], F32, tag="ps_a", name=f"psv{b}_{mt}_{dh}")
                    nc.tensor.matmul(
                        psv,
                        xT[:, mt * 128:(mt + 1) * 128],
                        wv_sb[:, dh * 512:(dh + 1) * 512],
                    )
                    # split the PSUM->SBUF evictions across DVE and ACT
                    if dh == 0:
                        nc.scalar.copy(out=vmt[:, 0:512], in_=psv)
                    else:
                        nc.vector.tensor_copy(out=vmt[:, 512:1024], in_=psv)
                vv.append(vmt)

            # ---------------- E = exp(distT + colmask), computed on the host ----------------
            E = [dcs[b][:, mt * N:(mt + 1) * N] for mt in range(NT)]

            # ---------------- per half: proj + scores + softmax numerator + rowsums ----------------
            p_half = []
            rB_half = []
            for hh in range(2):
                heads = range(hh * HH, (hh + 1) * HH)
                qT, kT = [], []
                for h in heads:
                    psq = ps_a.tile([128, N], F32, tag="ps_a", name=f"psq{b}_{h}")
                    nc.tensor.matmul(psq, wq_sb[:, h * D:(h + 1) * D], xT)
                    qTh = qkp.tile([128, N], BF16, tag="qT", name=f"qT{b}_{h}")
                    nc.scalar.activation(
                        out=qTh, in_=psq, func=mybir.ActivationFunctionType.Identity,
                        bias=bq_sb[:, h:h + 1],
                    )
                    qT.append(qTh)
                    psk = ps_a.tile([128, N], F32, tag="ps_a", name=f"psk{b}_{h}")
                    nc.tensor.matmul(psk, wk_sb[:, h * D:(h + 1) * D], xT)
                    # k-bias dropped: softmax over keys is invariant to the
                    # per-query shift bk.q it induces
                    kTh = qkp.tile([128, N], BF16, tag="kT", name=f"kT{b}_{h}")
                    if h % 4 == 0:
                        nc.scalar.copy(out=kTh, in_=psk)
                    else:
                        nc.vector.tensor_copy(out=kTh, in_=psk)
                    kT.append(kTh)

                p = [
                    ppool.tile([128, HH * N], BF16, tag="p", name=f"p{b}_{hh}_{mt}")
                    for mt in range(NT)
                ]
                rsrow = stage.tile([1, HH * N], F32, tag="rsrow", bufs=2, name=f"rsr{b}_{hh}")
                for j, h in enumerate(heads):
                    for mt in range(NT):
                        pss = ps_a.tile([128, N], F32, tag="ps_a", name=f"pss{b}_{h}_{mt}")
                        nc.tensor.matmul(pss, kT[j][:, mt * 128:(mt + 1) * 128], qT[j])
                        es = stage.tile([128, N], BF16, tag="exps", bufs=8, name=f"es{b}_{h}_{mt}")
                        nc.scalar.activation(
                            out=es, in_=pss, func=mybir.ActivationFunctionType.Exp
                        )
                        nc.vector.tensor_mul(
                            p[mt][:, j * N:(j + 1) * N], es, E[mt]
                        )
                    # rowsum for this head as soon as its p tiles are done
                    prs = ps_rs.tile([1, N], F32, tag="rs", name=f"prs{b}_{h}")
                    for mt in range(NT):
                        nc.tensor.matmul(
                            prs, ones_bf, p[mt][:, j * N:(j + 1) * N],
                            start=(mt == 0), stop=(mt == NT - 1),
                        )
                    # split rowsum evictions across ACT and DVE
                    if j % 2 == 0:
                        nc.scalar.copy(out=rsrow[0:1, j * N:(j + 1) * N], in_=prs)
                    else:
                        nc.vector.tensor_copy(out=rsrow[0:1, j * N:(j + 1) * N], in_=prs)
                p_half.append(p)

                # 1/rowsum chain (latency hidden under the following sections)
                rs4 = stage.tile([HH, N], F32, tag="rs4", bufs=2, name=f"rs4{b}_{hh}")
                nc.sync.dma_start(out=rs4, in_=rsrow.rearrange("o (h n) -> o h n", h=HH))
                rinv = stage.tile([HH, N], F32, tag="rinv", bufs=2, name=f"rinv{b}_{hh}")
                nc.vector.reciprocal_approx_fast(out=rinv, in_=rs4)
                rinvm = stage.tile([HH, N], BF16, tag="rinvm", bufs=2, name=f"rinvm{b}_{hh}")
                nc.vector.tensor_mul(rinvm, rinv, mask4.bitcast(F32))
                nc.sync.dma_start(out=rinv_scratch[b, hh * HH:(hh + 1) * HH, :], in_=rinvm)
                rB4 = rpool.tile([128, HH * N], BF16, tag="rB", name=f"rB{b}_{hh}")
                nc.sync.dma_start(
                    out=rB4,
                    in_=bcastP(
                        rinv_scratch[b, hh * HH:(hh + 1) * HH, :].rearrange("h n -> (h n)"),
                        128,
                    ),
                )
                rB_half.append(rB4)
            fronts.append((p_half, rB_half, vv, mask4))

        for b in range(BPC):
            p_half, rB_half, vv, mask4 = fronts[b]
            # ---------------- y + fused normalize-evict + output projection ----------------
            pso = ps_t.tile([128, N], F32, tag="pso", bufs=1, name=f"pso{b}")
            for hh in range(2):
                p = p_half[hh]
                rB4 = rB_half[hh]
                for j, h in enumerate(range(hh * HH, (hh + 1) * HH)):
                    py = ps_y.tile([128, N], F32, tag="ps_a", name=f"py{b}_{h}")
                    for mt in range(NT):
                        nc.tensor.matmul(
                            py,
                            vv[mt][:, h * D:(h + 1) * D],
                            p[mt][:, j * N:(j + 1) * N],
                            start=(mt == 0), stop=(mt == NT - 1),
                        )
                    yTn = ypool.tile([128, N], BF16, tag="yTn", name=f"yTn{b}_{h}")
                    nc.vector.tensor_mul(yTn, py, rB4[:, j * N:(j + 1) * N])
                    nc.tensor.matmul(
                        pso, wo_sb[:, h, :], yTn,
                        start=(h == 0), stop=False,
                    )
            nc.tensor.matmul(
                pso, bo_sb, mask4[0:1, :], start=False, stop=True
            )
            oT = stage.tile([128, N], F32, tag="oT")
            nc.scalar.copy(out=oT, in_=pso)

            # ---------------- transpose back to [n, d] and store (1 DMA) ----------------
            o_nat = stage.tile([128, NT, D], F32, tag="o_nat")
            for nt in range(NT):
                pst = ps_t.tile([128, 128], F32, tag="pst", name=f"pot{b}_{nt}")
                nc.tensor.transpose(pst, oT[:, nt * 128:(nt + 1) * 128], ident)
                nc.scalar.copy(out=o_nat[:, nt, :], in_=pst)
            nc.sync.dma_start(
                out=y_d[b].rearrange("(t p) d -> p t d", p=128), in_=o_nat
            )

    nc.compile()
    return nc


_NC_CACHE = None


def _get_nc():
    global _NC_CACHE
    if _NC_CACHE is None:
        _NC_CACHE = build_kernel()
    return _NC_CACHE


def kernel(x, dist, mask, Wq, bq, Wk, bk, Wv, bv, Wo, bo, **kw):
    from concourse.bass_utils import run_bass_kernel_spmd

    x = np.ascontiguousarray(np.asarray(x, dtype=np.float32))
    dist = np.ascontiguousarray(np.asarray(dist, dtype=np.float32))
    mask = np.ascontiguousarray(np.asarray(mask, dtype=np.float32))
    Wq = np.asarray(Wq, np.float32)
    Wk = np.asarray(Wk, np.float32)
    Wv = np.asarray(Wv, np.float32)
    Wo = np.asarray(Wo, np.float32)
    bq = np.asarray(bq, np.float32)
    bk = np.asarray(bk, np.float32)
    bv = np.asarray(bv, np.float32)
    bo = np.asarray(bo, np.float32)

    scale = np.float32(D) ** np.float32(-0.5)
    # wb blob [128, 4096] bf16: wq' | wk | wv | wo  (wo as [k, h, d_out])
    wo_r = Wo.reshape(H, D, D).transpose(1, 0, 2).reshape(D, H * D)
    wb = np.concatenate([Wq * scale, Wk, Wv, wo_r], axis=1).astype(ml_dtypes.bfloat16)
    # wf blob [128, 144] f32: bq' | bk | row0 = bo + bv @ Wo
    wf = np.zeros((D, 144), np.float32)
    wf[:, 0:8] = (bq * scale).reshape(H, D).T
    wf[:, 8:16] = bk.reshape(H, D).T
    wf[0, 16:144] = bo + bv @ Wo
    # xm [BPC, 128, 516] f32: x as [p, nt*d] | maskT
    xm = np.zeros((B, 128, 516), np.float32)
    xm[:, :, 0:512] = x.reshape(B, NT, 128, D).transpose(0, 2, 1, 3).reshape(B, 128, 512)
    xm[:, :, 512:516] = mask.reshape(B, NT, 128).transpose(0, 2, 1)
    # dc [B, 128, NT*N] bf16: dist transposed + column(key) mask, [m_in_tile, mt, n]
    cm = (mask - 1.0) * np.float32(1e9)  # [B, N] over keys m
    distT = dist.transpose(0, 2, 1) + cm[:, :, None]  # [B, m, n]
    # ship E = exp(distT + colmask) directly (bf16) -- no device exp needed
    eT = np.exp(np.minimum(distT, 80.0), dtype=np.float32)
    dc = np.ascontiguousarray(
        eT.reshape(B, NT, 128, N).transpose(0, 2, 1, 3).reshape(B, 128, NT * N)
    ).astype(ml_dtypes.bfloat16)

    nc = _get_nc()
    in_maps = []
    for c in range(NCORES):
        sl = slice(c * BPC, (c + 1) * BPC)
        in_maps.append(
            {
                "xm_in": np.ascontiguousarray(xm[sl]),
                "dc_in": np.ascontiguousarray(dc[sl]),
                "mask_in": mask[sl],
                "wb_in": wb,
                "wf_in": wf,
            }
        )
    res = run_bass_kernel_spmd(nc, in_maps, core_ids=list(range(NCORES)), **kw)
    global LAST_RESULT
    LAST_RESULT = res
    out = np.concatenate([res.results[c]["y_out"] for c in range(NCORES)], axis=0)
    return out


LAST_RESULT = None


if __name__ == "__main__":
    nc = build_kernel()
    print("kernel built ok")

